# revision 48
# baseline (speedup 1.0000x reference)
"""GPT-2 small (L=12, C=768, H=12, T=1024, B=4) forward on 8 trn2 NeuronCores.

Sharding: data-parallel over batch (4 elems) x 2-way vocab shard of lm_head.
Core c handles batch elem c//2, vocab half c%2. Zero collectives.

On-device layout: residual stream kept TRANSPOSED h_T [C(part), T(free)] as
6 tiles [128, 1024] f32. All matmuls contract over the partition dim; weights
stream from HBM in stationary blocks. LayerNorm stats via ones-vector matmuls
on a bf16 mirror; row-broadcasts (rstd, -mu*rstd, softmax 1/z) are done on
the PE with ones-row / selector stationaries instead of gpsimd. Attention
computes transposed scores (K stationary) in [128,1024] PSUM pairs (one exp
covers two key blocks); the softmax denominator comes from an extra ones
column appended per-head to V; z rows are collected into a [12,512] tile via
SBUF-to-SBUF DMA and inverted with ONE vector-engine reciprocal per token
group (no ACT table thrash - ACT only ever runs sqrt/exp/gelu/copy).
LayerNorm stats for the next phase are emitted under the current phase's
matmul stream so their cross-engine chain latency is hidden; attention items
are interleaved with QKV/V/proj matmul chunks so the PE never starves while
the scalar engine works through the softmax exponentials. MLP streams fc
weights once per layer (both token halves per block share one [128,1024]
PSUM pair drained by a single gelu). Logits are written bf16, upcast on host.
"""

import sys
import time
import numpy as np

for _p in ("/opt/trn_rl_repo", "/root/.axon_site/_ro/trn_rl_repo"):
    if _p not in sys.path:
        sys.path.insert(0, _p)

import ml_dtypes

BF16 = ml_dtypes.bfloat16

B, T, L, H, C = 4, 1024, 12, 12, 768
D = C // H
F = 4 * C
V = 50257
VPAD = 51200
VSH = VPAD // 2
CB = C // 128          # 6
FBL = F // 128         # 24
TT = T // 128          # 8
NTG = T // 512         # 2
NVC = VSH // 512       # 50
EPS = 1e-5

_CACHE = {}


def _build(ln_simple=True, reps=1):
    import concourse.bass as bass
    import concourse.mybir as mybir
    import concourse.tile as tile
    from concourse import bacc
    from contextlib import ExitStack

    f32 = mybir.dt.float32
    bf16 = mybir.dt.bfloat16
    AF = mybir.ActivationFunctionType
    ALU = mybir.AluOpType
    ds = bass.ds

    nc = bacc.Bacc("TRN2", target_bir_lowering=False, debug=False,
                   enable_asserts=False, num_devices=8)

    h0 = nc.dram_tensor("h0", [128, CB * T], f32, kind="ExternalInput").ap()
    wqk = nc.dram_tensor("wqk", [L, 128, 12 * CB * 128], bf16, kind="ExternalInput").ap()
    wv = nc.dram_tensor("wv", [L, 128, CB * 768], bf16, kind="ExternalInput").ap()
    wproj = nc.dram_tensor("wproj", [L, 128, CB * CB * 128], bf16, kind="ExternalInput").ap()
    wfc = nc.dram_tensor("wfc", [L, 128, FBL * CB * 128], bf16, kind="ExternalInput").ap()
    wmp = nc.dram_tensor("wmp", [L, 128, CB * FBL * 128], bf16, kind="ExternalInput").ap()
    wlm = nc.dram_tensor("wlm", [128, NVC * CB * 512], bf16, kind="ExternalInput").ap()
    lnp = nc.dram_tensor("lnp", [128, (4 * L + 2) * CB], f32, kind="ExternalInput").ap()
    msk = nc.dram_tensor("msk", [128, 4 * 512], bf16, kind="ExternalInput").ap()
    selc = nc.dram_tensor("selc", [12, CB * 128], bf16, kind="ExternalInput").ap()
    out = nc.dram_tensor("out", [T, VSH], bf16, kind="ExternalOutput").ap()

    ISD = float(1.0 / np.sqrt(D))

    with tile.TileContext(nc) as tc, ExitStack() as ctx:
        const = ctx.enter_context(tc.tile_pool(name="const", bufs=1))
        ph = ctx.enter_context(tc.tile_pool(name="ph", bufs=1))
        phb = ctx.enter_context(tc.tile_pool(name="phb", bufs=1))
        phn = ctx.enter_context(tc.tile_pool(name="phn", bufs=1))
        pbig = ctx.enter_context(tc.tile_pool(name="pbig", bufs=1))
        py = ctx.enter_context(tc.tile_pool(name="py", bufs=1))
        pexp = ctx.enter_context(tc.tile_pool(name="pexp", bufs=2))
        psq = ctx.enter_context(tc.tile_pool(name="psq", bufs=4))
        pab = ctx.enter_context(tc.tile_pool(name="pab", bufs=1))
        prowA = ctx.enter_context(tc.tile_pool(name="prowA", bufs=1))
        prowB = ctx.enter_context(tc.tile_pool(name="prowB", bufs=2))
        prowC = ctx.enter_context(tc.tile_pool(name="prowC", bufs=1))
        pwst = ctx.enter_context(tc.tile_pool(name="pwst", bufs=6))
        pwm = ctx.enter_context(tc.tile_pool(name="pwm", bufs=2))
        pwv = ctx.enter_context(tc.tile_pool(name="pwv", bufs=1))
        plm = ctx.enter_context(tc.tile_pool(name="plm", bufs=2))
        pout = ctx.enter_context(tc.tile_pool(name="pout", bufs=4))
        pwide = ctx.enter_context(tc.tile_pool(name="pwide", bufs=2, space="PSUM"))
        pmm = ctx.enter_context(tc.tile_pool(name="pmm", bufs=2, space="PSUM"))
        pst = ctx.enter_context(tc.tile_pool(name="pst", bufs=1, space="PSUM"))

        ones = const.tile([128, 1], bf16, tag="ones", name="ones")
        nc.vector.memset(ones[:], 1.0)
        onesr = const.tile([1, 128], bf16, tag="onesr", name="onesr")
        nc.vector.memset(onesr[:], 1.0)
        eps1 = const.tile([1, 1], f32, tag="eps1", name="eps1")
        nc.vector.memset(eps1[:], EPS)
        masks = const.tile([128, 4 * 512], bf16, tag="masks", name="masks")
        nc.sync.dma_start(masks[:], msk[:])
        lnt = const.tile([128, (4 * L + 2) * CB], f32, tag="lnt", name="lnt")
        nc.sync.dma_start(lnt[:], lnp[:])
        # sel[cb]: [12,128] selector - row 2cb -> out partitions 0:64,
        # row 2cb+1 -> out partitions 64:128 (per-head-pair broadcast)
        selall = const.tile([12, CB * 128], bf16, tag="selall", name="selall")
        nc.sync.dma_start(selall[:], selc[:])
        sel = [selall[:, ds(cb * 128, 128)] for cb in range(CB)]

        hT = [ph.tile([128, T], f32, tag=f"h{cb}", name=f"h{cb}") for cb in range(CB)]
        hbf = [phb.tile([128, T], bf16, tag=f"hb{cb}", name=f"hb{cb}") for cb in range(CB)]
        hn = [phn.tile([128, T], bf16, tag=f"hn{cb}", name=f"hn{cb}") for cb in range(CB)]
        yT = [py.tile([128, T], bf16, tag=f"y{i}", name=f"y{i}") for i in range(CB)]

        def big(i):
            return pbig.tile([128, T], bf16, tag=f"big{i}", name=f"big{i}")

        def ln_stats(tg):
            """Column sums of hbf and hbf^2 over C -> two [1,512] psum rows."""
            sl = ds(tg * 512, 512)
            st0 = pst.tile([128, 512], f32, tag="st0", name="st0")
            for cb in range(CB):
                nc.tensor.matmul(st0[0:1, :], ones[:], hbf[cb][:, sl],
                                 start=(cb == 0), stop=(cb == CB - 1))
            st1 = pst.tile([128, 512], f32, tag="st1", name="st1")
            for cb in range(CB):
                sq = psq.tile([128, 512], bf16, tag="sq", name="sq")
                nc.vector.tensor_mul(sq[:], hbf[cb][:, sl], hbf[cb][:, sl])
                nc.tensor.matmul(st1[0:1, :], ones[:], sq[:],
                                 start=(cb == 0), stop=(cb == CB - 1))
            return st0, st1

        def ln_rows(stats):
            """Row chain (DVE/ACT only - no PE instructions) so it can be
            emitted early and hide under an unrelated matmul stream."""
            st0, st1 = stats
            mu = prowA.tile([1, 512], f32, tag="mu", name="mu")
            nc.vector.tensor_scalar(mu[:], st0[0:1, :], 1.0 / C, None, ALU.mult)
            musq = prowA.tile([1, 512], f32, tag="std", name="musq")
            nc.vector.tensor_mul(musq[:], mu[:], mu[:])
            var = prowA.tile([1, 512], f32, tag="var", name="var")
            nc.vector.scalar_tensor_tensor(var[:], st1[0:1, :], 1.0 / C, musq[:],
                                           ALU.mult, ALU.subtract)
            std = prowA.tile([1, 512], f32, tag="std", name="std")
            nc.scalar.activation(std[:], var[:], AF.Sqrt, bias=eps1[:])
            rstd = prowA.tile([1, 512], f32, tag="rstd", name="rstd")
            nc.vector.reciprocal(rstd[:], std[:])
            rstdb = prowA.tile([1, 512], bf16, tag="rstdb", name="rstdb")
            nc.vector.tensor_copy(rstdb[:], rstd[:])
            nmr = prowA.tile([1, 512], bf16, tag="nmr", name="nmr")
            nc.vector.scalar_tensor_tensor(nmr[:], mu[:], -1.0, rstd[:],
                                           ALU.mult, ALU.mult)
            return rstdb, nmr

        def ln_apply(idx_w, idx_b, dst, tg, rows):
            rstdb, nmr = rows
            sl = ds(tg * 512, 512)
            abc = pmm.tile([128, 512], f32, tag="mm", name="abc")
            nc.tensor.matmul(abc[:], onesr[:], rstdb[:], start=True, stop=True)
            bbc = pmm.tile([128, 512], f32, tag="mm", name="bbc")
            nc.tensor.matmul(bbc[:], onesr[:], nmr[:], start=True, stop=True)
            abs_ = pab.tile([128, 512], bf16, tag="abc", name="abc_s")
            nc.vector.tensor_copy(abs_[:], abc[:])
            bbs = pab.tile([128, 512], bf16, tag="bbc", name="bbc_s")
            nc.vector.tensor_copy(bbs[:], bbc[:])
            for cb in range(CB):
                if ln_simple:
                    t1 = psq.tile([128, 512], bf16, tag="sq", name="t1")
                    nc.vector.tensor_mul(t1[:], hbf[cb][:, sl], abs_[:])
                    nc.vector.tensor_add(dst[cb][:, sl], t1[:], bbs[:])
                else:
                    t1 = psq.tile([128, 512], bf16, tag="sq", name="t1")
                    nc.vector.tensor_mul(t1[:], hbf[cb][:, sl], abs_[:])
                    nc.vector.tensor_add(t1[:], t1[:], bbs[:])
                    nc.vector.tensor_scalar(
                        dst[cb][:, sl], t1[:],
                        lnt[:, ds(idx_w * CB + cb, 1)],
                        lnt[:, ds(idx_b * CB + cb, 1)],
                        ALU.mult, ALU.add)

        def ln_finish(idx_w, idx_b, dst, tg, stats):
            ln_apply(idx_w, idx_b, dst, tg, ln_rows(stats))

        def qkv_db(l, tg, db, qT, kT):
            sl = ds(tg * 512, 512)
            wt = pwst.tile([128, 768], bf16, tag="wst", name="wst")
            nc.sync.dma_start(wt[:], wqk[l, :, ds(db * 768, 768)])
            ps = pmm.tile([128, 512], f32, tag="mm", name="mm")
            for cb in range(CB):
                nc.tensor.matmul(ps[:], wt[:, ds(cb * 128, 128)],
                                 hn[cb][:, sl],
                                 start=(cb == 0), stop=(cb == CB - 1))
            if db < 6:
                nc.scalar.activation(qT[db][:, sl], ps[:], AF.Copy, scale=ISD)
            else:
                nc.scalar.copy(kT[db - 6][:, sl], ps[:])

        def v_tt(tt, wvt, vA):
            va3 = vA[tt][:, 0:H * (D + 1)].rearrange("p (h e) -> p h e", e=D + 1)
            nc.vector.memset(va3[:, :, D:D + 1], 1.0)
            for half in range(2):
                w = 512 if half == 0 else 256
                nh = w // D
                ps = pmm.tile([128, 512], f32, tag="mm", name="mm")
                for cb in range(CB):
                    nc.tensor.matmul(ps[:, 0:w],
                                     hn[cb][:, ds(tt * 128, 128)],
                                     wvt[:, ds(cb * 768 + half * 512, w)],
                                     start=(cb == 0), stop=(cb == CB - 1))
                nc.vector.tensor_copy(
                    va3[:, ds(half * 8, nh), 0:D],
                    ps[:, 0:w].rearrange("p (h e) -> p h e", e=D))

        def att_a(hd, tg, qT, kT):
            po = (hd % 2) * 64
            qs = qT[hd // 2][po:po + 64, :]
            ks = kT[hd // 2][po:po + 64, :]
            npair = 2 * (tg + 1)
            ew = []
            for p in range(npair):
                wide = pwide.tile([128, 1024], f32, tag="aw", name="aw")
                for half in range(2):
                    sb = 2 * p + half
                    nc.tensor.matmul(wide[:, ds(half * 512, 512)],
                                     ks[:, ds(sb * 128, 128)],
                                     qs[:, ds(tg * 512, 512)],
                                     start=True, stop=True)
                e = pexp.tile([128, 1024], bf16, tag=f"e{p}", name=f"e{p}")
                nc.scalar.activation(e[:], wide[:], AF.Exp)
                kk0 = 2 * p - 4 * tg
                if kk0 >= 0:
                    nc.vector.tensor_mul(e[:], e[:],
                                         masks[:, ds(kk0 * 512, 1024)])
                ew.append(e)
            return ew

        def att_b(hd, tg, ew, vA, zsb):
            po = (hd % 2) * 64
            sl = ds(tg * 512, 512)
            nsb = 4 * (tg + 1)
            yps = pmm.tile([128, 512], f32, tag="mm", name="yps")
            for p, e in enumerate(ew):
                for half in range(2):
                    sb = 2 * p + half
                    nc.tensor.matmul(yps[0:65, :],
                                     vA[sb][:, ds(hd * 65, 65)],
                                     e[:, ds(half * 512, 512)],
                                     start=(sb == 0), stop=(sb == nsb - 1))
            ztmp = prowB.tile([1, 512], f32, tag="ztmp", name="ztmp")
            nc.vector.tensor_copy(ztmp[:], yps[64:65, :])
            nc.sync.dma_start(zsb[hd:hd + 1, :], ztmp[:])
            nc.vector.tensor_copy(yT[hd // 2][po:po + 64, sl], yps[0:64, :])

        def att_norm(tg, zsb):
            sl = ds(tg * 512, 512)
            zr = prowC.tile([12, 512], f32, tag="zr", name="zr")
            nc.vector.reciprocal(zr[:], zsb[:])
            zb = prowC.tile([12, 512], bf16, tag="zb", name="zb")
            nc.vector.tensor_copy(zb[:], zr[:])
            for cb in range(CB):
                izp = pmm.tile([128, 512], f32, tag="mm", name="izp")
                nc.tensor.matmul(izp[:], sel[cb], zb[:], start=True, stop=True)
                nc.vector.tensor_mul(yT[cb][:, sl], yT[cb][:, sl], izp[:])

        def proj_cb(l, tg, cb):
            sl = ds(tg * 512, 512)
            wt = pwst.tile([128, 768], bf16, tag="wst", name="wst")
            nc.sync.dma_start(wt[:], wproj[l, :, ds(cb * 768, 768)])
            ps = pmm.tile([128, 512], f32, tag="mm", name="mm")
            for k in range(CB):
                nc.tensor.matmul(ps[:], wt[:, ds(k * 128, 128)],
                                 yT[k][:, sl],
                                 start=(k == 0), stop=(k == CB - 1))
            nc.vector.tensor_add(hT[cb][:, sl], hT[cb][:, sl], ps[:])
            nc.vector.tensor_copy(hbf[cb][:, sl], hT[cb][:, sl])

        def fc_fb(l, fb, gl):
            wt = pwst.tile([128, 768], bf16, tag="wst", name="wst")
            nc.sync.dma_start(wt[:], wfc[l, :, ds(fb * 768, 768)])
            wide = pwide.tile([128, 1024], f32, tag="aw", name="fcw")
            for tg in range(NTG):
                for cb in range(CB):
                    nc.tensor.matmul(wide[:, ds(tg * 512, 512)],
                                     wt[:, ds(cb * 128, 128)],
                                     hn[cb][:, ds(tg * 512, 512)],
                                     start=(cb == 0), stop=(cb == CB - 1))
            nc.scalar.activation(gl[fb][:], wide[:], AF.Gelu_apprx_tanh)

        def fc_fb_half(l, fb, tg, gl):
            """Single-tg fc block: load wfc[fb], matmul one token group,
            gelu into gl[fb] half. Two passes per layer lets the tg0 pass
            serve as PE cover for the attention z-chain and LN2 rows."""
            wt = pwst.tile([128, 768], bf16, tag="wst", name="wst")
            nc.sync.dma_start(wt[:], wfc[l, :, ds(fb * 768, 768)])
            sl = ds(tg * 512, 512)
            ps = pmm.tile([128, 512], f32, tag="mm", name="mm")
            for cb in range(CB):
                nc.tensor.matmul(ps[:], wt[:, ds(cb * 128, 128)],
                                 hn[cb][:, sl],
                                 start=(cb == 0), stop=(cb == CB - 1))
            nc.scalar.activation(gl[fb][:, sl], ps[:], AF.Gelu_apprx_tanh)

        def mproj_cb(l, tg, cb, gl):
            sl = ds(tg * 512, 512)
            wt = pwm.tile([128, F], bf16, tag="wm", name="wm")
            nc.sync.dma_start(wt[:], wmp[l, :, ds(cb * F, F)])
            ps = pmm.tile([128, 512], f32, tag="mm", name="mm")
            for fb in range(FBL):
                nc.tensor.matmul(ps[:], wt[:, ds(fb * 128, 128)],
                                 gl[fb][:, sl],
                                 start=(fb == 0), stop=(fb == FBL - 1))
            nc.vector.tensor_add(hT[cb][:, sl], hT[cb][:, sl], ps[:])
            nc.vector.tensor_copy(hbf[cb][:, sl], hT[cb][:, sl])

        for rep in range(reps):
            for cb in range(CB):
                nc.sync.dma_start(hT[cb][:], h0[:, ds(cb * T, T)])
                nc.vector.tensor_copy(hbf[cb][:], hT[cb][:])
            # preamble: LN1(layer0) pipelined pieces
            stats0 = ln_stats(0)
            ln_finish(0, 1, hn, 0, stats0)
            stats1 = ln_stats(1)
            for l in range(L):
                # on entry: hn[:, tg0] applies emitted; LN1-tg1 stats in
                # `stats1`; its finish is emitted after qkv-tg0 so the row
                # chain hides under the qkv matmul stream.
                qT = [big(i) for i in range(CB)]
                kT = [big(6 + i) for i in range(CB)]
                vA = [big(12 + tt) for tt in range(TT)]
                rows1 = ln_rows(stats1)
                for db in range(12):
                    qkv_db(l, 0, db, qT, kT)
                ln_apply(4 * l + 0, 4 * l + 1, hn, 1, rows1)
                wvt = pwv.tile([128, CB * 768], bf16, tag="wv", name="wv")
                nc.sync.dma_start(wvt[:], wv[l, :, :])
                for tt in range(4):
                    v_tt(tt, wvt, vA)
                # attention tg0 interleaved with qkv-tg1 + V(4..7)
                bg = [lambda db=db: qkv_db(l, 1, db, qT, kT) for db in range(12)]
                bg += [lambda tt=tt: v_tt(tt, wvt, vA) for tt in range(4, 8)]
                zsb0 = prowB.tile([12, 512], f32, tag="zsb", name="zsb")
                prev = None
                for hd in range(H):
                    ew = att_a(hd, 0, qT, kT)
                    if hd >= 1 and bg:
                        bg.pop(0)()
                    if prev is not None:
                        att_b(prev[0], 0, prev[1], vA, zsb0)
                    prev = (hd, ew)
                att_b(prev[0], 0, prev[1], vA, zsb0)
                for f in bg:
                    f()
                # attention tg1 interleaved with znorm-tg0, proj-tg0, LN2-tg0
                stats20 = []
                bg = [lambda: att_norm(0, zsb0)]
                bg += [lambda cb=cb: proj_cb(l, 0, cb) for cb in range(CB)]
                bg += [lambda: stats20.append(ln_stats(0)),
                       lambda: ln_finish(4 * l + 2, 4 * l + 3, hn, 0,
                                         stats20[0])]
                zsb1 = prowB.tile([12, 512], f32, tag="zsb", name="zsb")
                prev = None
                for hd in range(H):
                    ew = att_a(hd, 1, qT, kT)
                    if hd >= 1 and bg:
                        bg.pop(0)()
                    if prev is not None:
                        att_b(prev[0], 1, prev[1], vA, zsb1)
                    prev = (hd, ew)
                att_b(prev[0], 1, prev[1], vA, zsb1)
                for f in bg:
                    f()
                # fc tg0 prefix covers the attention-tg1 z-chain and proj-tg1
                # waits; LN2-tg1 rows hide under the later fc-tg0 blocks, and
                # the remaining fc blocks run paired (both token groups, one
                # wide PSUM + one gelu) once hn-tg1 is ready.
                NSPLIT = 10
                gl = [big(i) for i in range(FBL)]
                for fb in range(4):
                    fc_fb_half(l, fb, 0, gl)
                att_norm(1, zsb1)
                for fb in range(4, 6):
                    fc_fb_half(l, fb, 0, gl)
                for cb in range(CB):
                    proj_cb(l, 1, cb)
                stats21 = ln_stats(1)
                rows21 = ln_rows(stats21)
                for fb in range(6, NSPLIT):
                    fc_fb_half(l, fb, 0, gl)
                ln_apply(4 * l + 2, 4 * l + 3, hn, 1, rows21)
                for fb in range(NSPLIT, FBL):
                    fc_fb(l, fb, gl)
                for fb in range(NSPLIT):
                    fc_fb_half(l, fb, 1, gl)
                # mproj + next layer's LN1 (or final LN) pipelined in;
                # the LN row chain (sqrt + slow 1-lane reciprocal) is emitted
                # before mproj-tg1 so it hides under those matmuls.
                last = (l == L - 1)
                nw = 4 * L if last else 4 * (l + 1)
                for cb in range(CB):
                    mproj_cb(l, 0, cb, gl)
                statsn0 = ln_stats(0)
                rowsn0 = ln_rows(statsn0)
                for cb in range(CB):
                    mproj_cb(l, 1, cb, gl)
                ln_apply(nw, nw + 1, hn, 0, rowsn0)
                stats1 = ln_stats(1)
            # final LN tg1 finish (stats already in stats1); prefetch the
            # first lm weight chunk so the head starts without a DMA wait
            rowsf1 = ln_rows(stats1)
            lt0 = plm.tile([128, CB * 512], bf16, tag="lm", name="lm")
            nc.sync.dma_start(lt0[:], wlm[:, ds(0, CB * 512)])
            ln_apply(4 * L, 4 * L + 1, hn, 1, rowsf1)

        # ---- lm head (hf = hn holds the final layernormed hidden) ----
        hf = hn
        for vc in range(NVC):
            if vc == 0:
                lt = lt0
            else:
                lt = plm.tile([128, CB * 512], bf16, tag="lm", name="lm")
                nc.sync.dma_start(lt[:], wlm[:, ds(vc * CB * 512, CB * 512)])
            for tt in range(TT):
                ps = pmm.tile([128, 512], f32, tag="mm", name="mm")
                for cb in range(CB):
                    nc.tensor.matmul(ps[:], hf[cb][:, ds(tt * 128, 128)],
                                     lt[:, ds(cb * 512, 512)],
                                     start=(cb == 0), stop=(cb == CB - 1))
                ot = pout.tile([128, 512], bf16, tag="ot", name="ot")
                nc.scalar.copy(ot[:], ps[:])
                nc.scalar.dma_start(out[ds(tt * 128, 128), ds(vc * 512, 512)],
                                    ot[:])

    nc.compile()
    return nc


def _pack_stationary(w, nblk):
    kb = w.shape[0] // 128
    t = w.reshape(kb, 128, nblk, 128)
    return np.ascontiguousarray(
        t.transpose(1, 2, 0, 3).reshape(128, nblk * kb * 128))


def _prep(inputs):
    wte = np.asarray(inputs["wte"], np.float32)
    wpe = np.asarray(inputs["wpe"], np.float32)
    x = np.asarray(inputs["x"])
    aw = np.asarray(inputs["attn_w"], np.float32)
    pw = np.asarray(inputs["attnp_w"], np.float32)
    fw = np.asarray(inputs["fc_w"], np.float32)
    mw = np.asarray(inputs["mproj_w"], np.float32)
    lm = np.asarray(inputs["lm_w"], np.float32)
    for nm in ("attn_b", "attnp_b", "fc_b", "mproj_b"):
        assert not np.any(np.asarray(inputs[nm])), f"{nm} nonzero; unsupported"

    ln_simple = all(
        np.all(np.asarray(inputs[nm], np.float32) == v)
        for nm, v in (("ln1_w", 1.0), ("ln2_w", 1.0), ("lnf_w", 1.0),
                      ("ln1_b", 0.0), ("ln2_b", 0.0), ("lnf_b", 0.0)))

    wqk = np.stack([_pack_stationary(aw[l][:, :2 * C], 12) for l in range(L)]).astype(BF16)
    wv = np.stack([np.ascontiguousarray(
        aw[l][:, 2 * C:].reshape(CB, 128, C).transpose(1, 0, 2).reshape(128, CB * C))
        for l in range(L)]).astype(BF16)
    wproj = np.stack([_pack_stationary(pw[l], CB) for l in range(L)]).astype(BF16)
    wfc = np.stack([_pack_stationary(fw[l], FBL) for l in range(L)]).astype(BF16)
    wmp = np.stack([_pack_stationary(mw[l], CB) for l in range(L)]).astype(BF16)

    lmp = np.zeros((C, VPAD), np.float32)
    lmp[:, :V] = lm
    wlm_halves = []
    for vh in range(2):
        t = lmp[:, vh * VSH:(vh + 1) * VSH].reshape(CB, 128, NVC, 512)
        wlm_halves.append(np.ascontiguousarray(
            t.transpose(1, 2, 0, 3).reshape(128, NVC * CB * 512)).astype(BF16))

    lncols = np.zeros((128, (4 * L + 2) * CB), np.float32)
    names = [("ln1_w", 0), ("ln1_b", 1), ("ln2_w", 2), ("ln2_b", 3)]
    for l in range(L):
        for nm, k in names:
            vec = np.asarray(inputs[nm], np.float32)[l]
            lncols[:, (4 * l + k) * CB:(4 * l + k + 1) * CB] = \
                vec.reshape(CB, 128).T
    lncols[:, 4 * L * CB:(4 * L + 1) * CB] = \
        np.asarray(inputs["lnf_w"], np.float32).reshape(CB, 128).T
    lncols[:, (4 * L + 1) * CB:] = \
        np.asarray(inputs["lnf_b"], np.float32).reshape(CB, 128).T

    p = np.arange(128)[:, None]
    f = np.arange(512)[None, :]
    masks = np.concatenate(
        [(f >= 128 * k + p).astype(np.float32) for k in range(4)],
        axis=1).astype(BF16)

    selc = np.zeros((12, CB * 128), np.float32)
    for cb in range(CB):
        selc[2 * cb, cb * 128:cb * 128 + 64] = 1.0
        selc[2 * cb + 1, cb * 128 + 64:cb * 128 + 128] = 1.0
    selc = selc.astype(BF16)

    h0s = []
    for b in range(B):
        h = wte[x[b]] + wpe[:T]
        hTr = np.ascontiguousarray(
            h.T.reshape(CB, 128, T).transpose(1, 0, 2).reshape(128, CB * T))
        h0s.append(hTr.astype(np.float32))

    in_maps = []
    for c in range(8):
        in_maps.append({
            "h0": h0s[c // 2], "wqk": wqk, "wv": wv, "wproj": wproj,
            "wfc": wfc, "wmp": wmp, "wlm": wlm_halves[c % 2],
            "lnp": lncols, "msk": masks, "selc": selc,
        })
    return in_maps, ln_simple


def kernel(**inputs):
    from concourse import bass_utils
    in_maps, ln_simple = _prep(inputs)
    key = ("nc", ln_simple)
    if key not in _CACHE:
        t0 = time.time()
        _CACHE[key] = _build(ln_simple=ln_simple)
        print(f"[kernel] build+compile {time.time()-t0:.1f}s", file=sys.stderr)
    nc = _CACHE[key]
    res = bass_utils.run_bass_kernel_spmd(nc, in_maps, core_ids=list(range(8)))
    outs = [r["out"].astype(np.float32) for r in res.results]
    full = np.empty((B, T, V), np.float32)
    for b in range(B):
        full[b] = np.concatenate([outs[2 * b], outs[2 * b + 1]], axis=1)[:, :V]
    return full


# revision 49
# speedup vs baseline: 1.0034x; 1.0034x over previous
"""GPT-2 small (L=12, C=768, H=12, T=1024, B=4) forward on 8 trn2 NeuronCores.

Sharding: data-parallel over batch (4 elems) x 2-way vocab shard of lm_head.
Core c handles batch elem c//2, vocab half c%2. Zero collectives.

On-device layout: residual stream kept TRANSPOSED h_T [C(part), T(free)] as
6 tiles [128, 1024] f32. All matmuls contract over the partition dim; weights
stream from HBM in stationary blocks. LayerNorm stats via ones-vector matmuls
on a bf16 mirror; row-broadcasts (rstd, -mu*rstd, softmax 1/z) are done on
the PE with ones-row / selector stationaries instead of gpsimd. Attention
computes transposed scores (K stationary) in [128,1024] PSUM pairs (one exp
covers two key blocks); the softmax denominator comes from an extra ones
column appended per-head to V; z rows are collected into a [12,512] tile via
SBUF-to-SBUF DMA and inverted with ONE vector-engine reciprocal per token
group (no ACT table thrash - ACT only ever runs sqrt/exp/gelu/copy).
LayerNorm stats for the next phase are emitted under the current phase's
matmul stream so their cross-engine chain latency is hidden; attention items
are interleaved with QKV/V/proj matmul chunks so the PE never starves while
the scalar engine works through the softmax exponentials. MLP streams fc
weights once per layer (both token halves per block share one [128,1024]
PSUM pair drained by a single gelu). Logits are written bf16, upcast on host.
"""

import sys
import time
import numpy as np

for _p in ("/opt/trn_rl_repo", "/root/.axon_site/_ro/trn_rl_repo"):
    if _p not in sys.path:
        sys.path.insert(0, _p)

import ml_dtypes

BF16 = ml_dtypes.bfloat16

B, T, L, H, C = 4, 1024, 12, 12, 768
D = C // H
F = 4 * C
V = 50257
VPAD = 51200
VSH = VPAD // 2
CB = C // 128          # 6
FBL = F // 128         # 24
TT = T // 128          # 8
NTG = T // 512         # 2
NVC = VSH // 512       # 50
EPS = 1e-5

_CACHE = {}


def _build(ln_simple=True, reps=1):
    import concourse.bass as bass
    import concourse.mybir as mybir
    import concourse.tile as tile
    from concourse import bacc
    from contextlib import ExitStack

    f32 = mybir.dt.float32
    bf16 = mybir.dt.bfloat16
    AF = mybir.ActivationFunctionType
    ALU = mybir.AluOpType
    ds = bass.ds

    nc = bacc.Bacc("TRN2", target_bir_lowering=False, debug=False,
                   enable_asserts=False, num_devices=8)

    h0 = nc.dram_tensor("h0", [128, CB * T], f32, kind="ExternalInput").ap()
    wqk = nc.dram_tensor("wqk", [L, 128, 12 * CB * 128], bf16, kind="ExternalInput").ap()
    wv = nc.dram_tensor("wv", [L, 128, CB * 768], bf16, kind="ExternalInput").ap()
    wproj = nc.dram_tensor("wproj", [L, 128, CB * CB * 128], bf16, kind="ExternalInput").ap()
    wfc = nc.dram_tensor("wfc", [L, 128, FBL * CB * 128], bf16, kind="ExternalInput").ap()
    wmp = nc.dram_tensor("wmp", [L, 128, CB * FBL * 128], bf16, kind="ExternalInput").ap()
    wlm = nc.dram_tensor("wlm", [128, NVC * CB * 512], bf16, kind="ExternalInput").ap()
    lnp = nc.dram_tensor("lnp", [128, (4 * L + 2) * CB], f32, kind="ExternalInput").ap()
    msk = nc.dram_tensor("msk", [128, 4 * 512], bf16, kind="ExternalInput").ap()
    selc = nc.dram_tensor("selc", [12, CB * 128], bf16, kind="ExternalInput").ap()
    out = nc.dram_tensor("out", [T, VSH], bf16, kind="ExternalOutput").ap()

    ISD = float(1.0 / np.sqrt(D))

    with tile.TileContext(nc) as tc, ExitStack() as ctx:
        const = ctx.enter_context(tc.tile_pool(name="const", bufs=1))
        ph = ctx.enter_context(tc.tile_pool(name="ph", bufs=1))
        phb = ctx.enter_context(tc.tile_pool(name="phb", bufs=1))
        phn = ctx.enter_context(tc.tile_pool(name="phn", bufs=1))
        pbig = ctx.enter_context(tc.tile_pool(name="pbig", bufs=1))
        py = ctx.enter_context(tc.tile_pool(name="py", bufs=1))
        pexp = ctx.enter_context(tc.tile_pool(name="pexp", bufs=2))
        psq = ctx.enter_context(tc.tile_pool(name="psq", bufs=4))
        pab = ctx.enter_context(tc.tile_pool(name="pab", bufs=1))
        prowA = ctx.enter_context(tc.tile_pool(name="prowA", bufs=1))
        prowB = ctx.enter_context(tc.tile_pool(name="prowB", bufs=2))
        prowC = ctx.enter_context(tc.tile_pool(name="prowC", bufs=1))
        pwst = ctx.enter_context(tc.tile_pool(name="pwst", bufs=6))
        pwm = ctx.enter_context(tc.tile_pool(name="pwm", bufs=2))
        pwv = ctx.enter_context(tc.tile_pool(name="pwv", bufs=1))
        plm = ctx.enter_context(tc.tile_pool(name="plm", bufs=2))
        pout = ctx.enter_context(tc.tile_pool(name="pout", bufs=4))
        pwide = ctx.enter_context(tc.tile_pool(name="pwide", bufs=2, space="PSUM"))
        pmm = ctx.enter_context(tc.tile_pool(name="pmm", bufs=2, space="PSUM"))
        pst = ctx.enter_context(tc.tile_pool(name="pst", bufs=1, space="PSUM"))

        ones = const.tile([128, 1], bf16, tag="ones", name="ones")
        nc.vector.memset(ones[:], 1.0)
        onesr = const.tile([1, 128], bf16, tag="onesr", name="onesr")
        nc.vector.memset(onesr[:], 1.0)
        eps1 = const.tile([1, 1], f32, tag="eps1", name="eps1")
        nc.vector.memset(eps1[:], EPS)
        masks = const.tile([128, 4 * 512], bf16, tag="masks", name="masks")
        nc.sync.dma_start(masks[:], msk[:])
        lnt = const.tile([128, (4 * L + 2) * CB], f32, tag="lnt", name="lnt")
        nc.sync.dma_start(lnt[:], lnp[:])
        # sel[cb]: [12,128] selector - row 2cb -> out partitions 0:64,
        # row 2cb+1 -> out partitions 64:128 (per-head-pair broadcast)
        selall = const.tile([12, CB * 128], bf16, tag="selall", name="selall")
        nc.sync.dma_start(selall[:], selc[:])
        sel = [selall[:, ds(cb * 128, 128)] for cb in range(CB)]

        hT = [ph.tile([128, T], f32, tag=f"h{cb}", name=f"h{cb}") for cb in range(CB)]
        hbf = [phb.tile([128, T], bf16, tag=f"hb{cb}", name=f"hb{cb}") for cb in range(CB)]
        hn = [phn.tile([128, T], bf16, tag=f"hn{cb}", name=f"hn{cb}") for cb in range(CB)]
        yT = [py.tile([128, T], bf16, tag=f"y{i}", name=f"y{i}") for i in range(CB)]

        def big(i):
            return pbig.tile([128, T], bf16, tag=f"big{i}", name=f"big{i}")

        def ln_stats(tg):
            """Column sums of hbf and hbf^2 over C -> two [1,512] psum rows."""
            sl = ds(tg * 512, 512)
            st0 = pst.tile([128, 512], f32, tag="st0", name="st0")
            for cb in range(CB):
                nc.tensor.matmul(st0[0:1, :], ones[:], hbf[cb][:, sl],
                                 start=(cb == 0), stop=(cb == CB - 1))
            st1 = pst.tile([128, 512], f32, tag="st1", name="st1")
            for cb in range(CB):
                sq = psq.tile([128, 512], bf16, tag="sq", name="sq")
                nc.vector.tensor_mul(sq[:], hbf[cb][:, sl], hbf[cb][:, sl])
                nc.tensor.matmul(st1[0:1, :], ones[:], sq[:],
                                 start=(cb == 0), stop=(cb == CB - 1))
            return st0, st1

        def ln_rows(stats):
            """Row chain (DVE/ACT only - no PE instructions) so it can be
            emitted early and hide under an unrelated matmul stream."""
            st0, st1 = stats
            mu = prowA.tile([1, 512], f32, tag="mu", name="mu")
            nc.vector.tensor_scalar(mu[:], st0[0:1, :], 1.0 / C, None, ALU.mult)
            musq = prowA.tile([1, 512], f32, tag="std", name="musq")
            nc.vector.tensor_mul(musq[:], mu[:], mu[:])
            var = prowA.tile([1, 512], f32, tag="var", name="var")
            nc.vector.scalar_tensor_tensor(var[:], st1[0:1, :], 1.0 / C, musq[:],
                                           ALU.mult, ALU.subtract)
            std = prowA.tile([1, 512], f32, tag="std", name="std")
            nc.scalar.activation(std[:], var[:], AF.Sqrt, bias=eps1[:])
            rstd = prowA.tile([1, 512], f32, tag="rstd", name="rstd")
            nc.vector.reciprocal(rstd[:], std[:])
            rstdb = prowA.tile([1, 512], bf16, tag="rstdb", name="rstdb")
            nc.vector.tensor_copy(rstdb[:], rstd[:])
            nmr = prowA.tile([1, 512], bf16, tag="nmr", name="nmr")
            nc.vector.scalar_tensor_tensor(nmr[:], mu[:], -1.0, rstd[:],
                                           ALU.mult, ALU.mult)
            return rstdb, nmr

        def ln_apply(idx_w, idx_b, dst, tg, rows):
            rstdb, nmr = rows
            sl = ds(tg * 512, 512)
            abc = pmm.tile([128, 512], f32, tag="mm", name="abc")
            nc.tensor.matmul(abc[:], onesr[:], rstdb[:], start=True, stop=True)
            bbc = pmm.tile([128, 512], f32, tag="mm", name="bbc")
            nc.tensor.matmul(bbc[:], onesr[:], nmr[:], start=True, stop=True)
            abs_ = pab.tile([128, 512], bf16, tag="abc", name="abc_s")
            nc.vector.tensor_copy(abs_[:], abc[:])
            bbs = pab.tile([128, 512], bf16, tag="bbc", name="bbc_s")
            nc.vector.tensor_copy(bbs[:], bbc[:])
            for cb in range(CB):
                if ln_simple:
                    t1 = psq.tile([128, 512], bf16, tag="sq", name="t1")
                    nc.vector.tensor_mul(t1[:], hbf[cb][:, sl], abs_[:])
                    nc.vector.tensor_add(dst[cb][:, sl], t1[:], bbs[:])
                else:
                    t1 = psq.tile([128, 512], bf16, tag="sq", name="t1")
                    nc.vector.tensor_mul(t1[:], hbf[cb][:, sl], abs_[:])
                    nc.vector.tensor_add(t1[:], t1[:], bbs[:])
                    nc.vector.tensor_scalar(
                        dst[cb][:, sl], t1[:],
                        lnt[:, ds(idx_w * CB + cb, 1)],
                        lnt[:, ds(idx_b * CB + cb, 1)],
                        ALU.mult, ALU.add)

        def ln_finish(idx_w, idx_b, dst, tg, stats):
            ln_apply(idx_w, idx_b, dst, tg, ln_rows(stats))

        def qkv_db(l, tg, db, qT, kT):
            sl = ds(tg * 512, 512)
            wt = pwst.tile([128, 768], bf16, tag="wst", name="wst")
            nc.sync.dma_start(wt[:], wqk[l, :, ds(db * 768, 768)])
            ps = pmm.tile([128, 512], f32, tag="mm", name="mm")
            for cb in range(CB):
                nc.tensor.matmul(ps[:], wt[:, ds(cb * 128, 128)],
                                 hn[cb][:, sl],
                                 start=(cb == 0), stop=(cb == CB - 1))
            if db < 6:
                nc.scalar.activation(qT[db][:, sl], ps[:], AF.Copy, scale=ISD)
            else:
                nc.scalar.copy(kT[db - 6][:, sl], ps[:])

        def v_tt(tt, wvt, vA):
            va3 = vA[tt][:, 0:H * (D + 1)].rearrange("p (h e) -> p h e", e=D + 1)
            nc.vector.memset(va3[:, :, D:D + 1], 1.0)
            for half in range(2):
                w = 512 if half == 0 else 256
                nh = w // D
                ps = pmm.tile([128, 512], f32, tag="mm", name="mm")
                for cb in range(CB):
                    nc.tensor.matmul(ps[:, 0:w],
                                     hn[cb][:, ds(tt * 128, 128)],
                                     wvt[:, ds(cb * 768 + half * 512, w)],
                                     start=(cb == 0), stop=(cb == CB - 1))
                nc.vector.tensor_copy(
                    va3[:, ds(half * 8, nh), 0:D],
                    ps[:, 0:w].rearrange("p (h e) -> p h e", e=D))

        def att_a(hd, tg, qT, kT):
            po = (hd % 2) * 64
            qs = qT[hd // 2][po:po + 64, :]
            ks = kT[hd // 2][po:po + 64, :]
            npair = 2 * (tg + 1)
            ew = []
            for p in range(npair):
                wide = pwide.tile([128, 1024], f32, tag="aw", name="aw")
                for half in range(2):
                    sb = 2 * p + half
                    nc.tensor.matmul(wide[:, ds(half * 512, 512)],
                                     ks[:, ds(sb * 128, 128)],
                                     qs[:, ds(tg * 512, 512)],
                                     start=True, stop=True)
                e = pexp.tile([128, 1024], bf16, tag=f"e{p}", name=f"e{p}")
                nc.scalar.activation(e[:], wide[:], AF.Exp)
                kk0 = 2 * p - 4 * tg
                if kk0 >= 0:
                    nc.vector.tensor_mul(e[:], e[:],
                                         masks[:, ds(kk0 * 512, 1024)])
                ew.append(e)
            return ew

        def att_b(hd, tg, ew, vA, zsb):
            po = (hd % 2) * 64
            sl = ds(tg * 512, 512)
            nsb = 4 * (tg + 1)
            yps = pmm.tile([128, 512], f32, tag="mm", name="yps")
            for p, e in enumerate(ew):
                for half in range(2):
                    sb = 2 * p + half
                    nc.tensor.matmul(yps[0:65, :],
                                     vA[sb][:, ds(hd * 65, 65)],
                                     e[:, ds(half * 512, 512)],
                                     start=(sb == 0), stop=(sb == nsb - 1))
            ztmp = prowB.tile([1, 512], f32, tag="ztmp", name="ztmp")
            nc.vector.tensor_copy(ztmp[:], yps[64:65, :])
            nc.sync.dma_start(zsb[hd:hd + 1, :], ztmp[:])
            nc.vector.tensor_copy(yT[hd // 2][po:po + 64, sl], yps[0:64, :])

        def att_norm(tg, zsb):
            sl = ds(tg * 512, 512)
            zr = prowC.tile([12, 512], f32, tag="zr", name="zr")
            nc.vector.reciprocal(zr[:], zsb[:])
            zb = prowC.tile([12, 512], bf16, tag="zb", name="zb")
            nc.vector.tensor_copy(zb[:], zr[:])
            for cb in range(CB):
                izp = pmm.tile([128, 512], f32, tag="mm", name="izp")
                nc.tensor.matmul(izp[:], sel[cb], zb[:], start=True, stop=True)
                nc.vector.tensor_mul(yT[cb][:, sl], yT[cb][:, sl], izp[:])

        def proj_cb(l, tg, cb):
            sl = ds(tg * 512, 512)
            wt = pwst.tile([128, 768], bf16, tag="wst", name="wst")
            nc.sync.dma_start(wt[:], wproj[l, :, ds(cb * 768, 768)])
            ps = pmm.tile([128, 512], f32, tag="mm", name="mm")
            for k in range(CB):
                nc.tensor.matmul(ps[:], wt[:, ds(k * 128, 128)],
                                 yT[k][:, sl],
                                 start=(k == 0), stop=(k == CB - 1))
            nc.vector.tensor_add(hT[cb][:, sl], hT[cb][:, sl], ps[:])
            nc.vector.tensor_copy(hbf[cb][:, sl], hT[cb][:, sl])

        def fc_fb(l, fb, gl):
            wt = pwst.tile([128, 768], bf16, tag="wst", name="wst")
            nc.sync.dma_start(wt[:], wfc[l, :, ds(fb * 768, 768)])
            wide = pwide.tile([128, 1024], f32, tag="aw", name="fcw")
            for tg in range(NTG):
                for cb in range(CB):
                    nc.tensor.matmul(wide[:, ds(tg * 512, 512)],
                                     wt[:, ds(cb * 128, 128)],
                                     hn[cb][:, ds(tg * 512, 512)],
                                     start=(cb == 0), stop=(cb == CB - 1))
            nc.scalar.activation(gl[fb][:], wide[:], AF.Gelu_apprx_tanh)

        def fc_fb_half(l, fb, tg, gl):
            """Single-tg fc block: load wfc[fb], matmul one token group,
            gelu into gl[fb] half. Two passes per layer lets the tg0 pass
            serve as PE cover for the attention z-chain and LN2 rows."""
            wt = pwst.tile([128, 768], bf16, tag="wst", name="wst")
            nc.sync.dma_start(wt[:], wfc[l, :, ds(fb * 768, 768)])
            sl = ds(tg * 512, 512)
            ps = pmm.tile([128, 512], f32, tag="mm", name="mm")
            for cb in range(CB):
                nc.tensor.matmul(ps[:], wt[:, ds(cb * 128, 128)],
                                 hn[cb][:, sl],
                                 start=(cb == 0), stop=(cb == CB - 1))
            nc.scalar.activation(gl[fb][:, sl], ps[:], AF.Gelu_apprx_tanh)

        def mproj_cb(l, tg, cb, gl):
            sl = ds(tg * 512, 512)
            wt = pwm.tile([128, F], bf16, tag="wm", name="wm")
            nc.sync.dma_start(wt[:], wmp[l, :, ds(cb * F, F)])
            ps = pmm.tile([128, 512], f32, tag="mm", name="mm")
            for fb in range(FBL):
                nc.tensor.matmul(ps[:], wt[:, ds(fb * 128, 128)],
                                 gl[fb][:, sl],
                                 start=(fb == 0), stop=(fb == FBL - 1))
            nc.vector.tensor_add(hT[cb][:, sl], hT[cb][:, sl], ps[:])
            nc.vector.tensor_copy(hbf[cb][:, sl], hT[cb][:, sl])

        for rep in range(reps):
            for cb in range(CB):
                nc.sync.dma_start(hT[cb][:], h0[:, ds(cb * T, T)])
                nc.vector.tensor_copy(hbf[cb][:], hT[cb][:])
            # preamble: LN1(layer0) pipelined pieces
            stats0 = ln_stats(0)
            ln_finish(0, 1, hn, 0, stats0)
            stats1 = ln_stats(1)
            for l in range(L):
                # on entry: hn[:, tg0] applies emitted; LN1-tg1 stats in
                # `stats1`; its finish is emitted after qkv-tg0 so the row
                # chain hides under the qkv matmul stream.
                qT = [big(i) for i in range(CB)]
                kT = [big(6 + i) for i in range(CB)]
                vA = [big(12 + tt) for tt in range(TT)]
                for db in range(12):
                    qkv_db(l, 0, db, qT, kT)
                ln_finish(4 * l + 0, 4 * l + 1, hn, 1, stats1)
                wvt = pwv.tile([128, CB * 768], bf16, tag="wv", name="wv")
                nc.sync.dma_start(wvt[:], wv[l, :, :])
                for tt in range(4):
                    v_tt(tt, wvt, vA)
                # attention tg0 interleaved with qkv-tg1 + V(4..7)
                bg = [lambda db=db: qkv_db(l, 1, db, qT, kT) for db in range(12)]
                bg += [lambda tt=tt: v_tt(tt, wvt, vA) for tt in range(4, 8)]
                zsb0 = prowB.tile([12, 512], f32, tag="zsb", name="zsb")
                prev = None
                for hd in range(H):
                    ew = att_a(hd, 0, qT, kT)
                    if hd >= 1 and bg:
                        bg.pop(0)()
                    if prev is not None:
                        att_b(prev[0], 0, prev[1], vA, zsb0)
                    prev = (hd, ew)
                att_b(prev[0], 0, prev[1], vA, zsb0)
                for f in bg:
                    f()
                # attention tg1 interleaved with znorm-tg0, proj-tg0, LN2-tg0
                stats20 = []
                bg = [lambda: att_norm(0, zsb0)]
                bg += [lambda cb=cb: proj_cb(l, 0, cb) for cb in range(CB)]
                bg += [lambda: stats20.append(ln_stats(0)),
                       lambda: ln_finish(4 * l + 2, 4 * l + 3, hn, 0,
                                         stats20[0])]
                zsb1 = prowB.tile([12, 512], f32, tag="zsb", name="zsb")
                prev = None
                for hd in range(H):
                    ew = att_a(hd, 1, qT, kT)
                    if hd >= 1 and bg:
                        bg.pop(0)()
                    if prev is not None:
                        att_b(prev[0], 1, prev[1], vA, zsb1)
                    prev = (hd, ew)
                att_b(prev[0], 1, prev[1], vA, zsb1)
                for f in bg:
                    f()
                # fc tg0 prefix covers the attention-tg1 z-chain and proj-tg1
                # waits; LN2-tg1 rows hide under the later fc-tg0 blocks, and
                # the remaining fc blocks run paired (both token groups, one
                # wide PSUM + one gelu) once hn-tg1 is ready.
                NSPLIT = 10
                gl = [big(i) for i in range(FBL)]
                for fb in range(4):
                    fc_fb_half(l, fb, 0, gl)
                att_norm(1, zsb1)
                for fb in range(4, 6):
                    fc_fb_half(l, fb, 0, gl)
                for cb in range(CB):
                    proj_cb(l, 1, cb)
                stats21 = ln_stats(1)
                rows21 = ln_rows(stats21)
                for fb in range(6, NSPLIT):
                    fc_fb_half(l, fb, 0, gl)
                ln_apply(4 * l + 2, 4 * l + 3, hn, 1, rows21)
                for fb in range(NSPLIT, FBL):
                    fc_fb(l, fb, gl)
                for fb in range(NSPLIT):
                    fc_fb_half(l, fb, 1, gl)
                # mproj + next layer's LN1 (or final LN) pipelined in;
                # the LN row chain (sqrt + slow 1-lane reciprocal) is emitted
                # before mproj-tg1 so it hides under those matmuls.
                last = (l == L - 1)
                nw = 4 * L if last else 4 * (l + 1)
                for cb in range(CB):
                    mproj_cb(l, 0, cb, gl)
                statsn0 = ln_stats(0)
                rowsn0 = ln_rows(statsn0)
                for cb in range(CB):
                    mproj_cb(l, 1, cb, gl)
                ln_apply(nw, nw + 1, hn, 0, rowsn0)
                stats1 = ln_stats(1)
            # final LN tg1 finish (stats already in stats1)
            ln_finish(4 * L, 4 * L + 1, hn, 1, stats1)

        # ---- lm head (hf = hn holds the final layernormed hidden) ----
        hf = hn
        for vc in range(NVC):
            lt = plm.tile([128, CB * 512], bf16, tag="lm", name="lm")
            nc.sync.dma_start(lt[:], wlm[:, ds(vc * CB * 512, CB * 512)])
            for tt in range(TT):
                ps = pmm.tile([128, 512], f32, tag="mm", name="mm")
                for cb in range(CB):
                    nc.tensor.matmul(ps[:], hf[cb][:, ds(tt * 128, 128)],
                                     lt[:, ds(cb * 512, 512)],
                                     start=(cb == 0), stop=(cb == CB - 1))
                ot = pout.tile([128, 512], bf16, tag="ot", name="ot")
                nc.scalar.copy(ot[:], ps[:])
                nc.scalar.dma_start(out[ds(tt * 128, 128), ds(vc * 512, 512)],
                                    ot[:])

    nc.compile()
    return nc


def _pack_stationary(w, nblk):
    kb = w.shape[0] // 128
    t = w.reshape(kb, 128, nblk, 128)
    return np.ascontiguousarray(
        t.transpose(1, 2, 0, 3).reshape(128, nblk * kb * 128))


def _prep(inputs):
    wte = np.asarray(inputs["wte"], np.float32)
    wpe = np.asarray(inputs["wpe"], np.float32)
    x = np.asarray(inputs["x"])
    aw = np.asarray(inputs["attn_w"], np.float32)
    pw = np.asarray(inputs["attnp_w"], np.float32)
    fw = np.asarray(inputs["fc_w"], np.float32)
    mw = np.asarray(inputs["mproj_w"], np.float32)
    lm = np.asarray(inputs["lm_w"], np.float32)
    for nm in ("attn_b", "attnp_b", "fc_b", "mproj_b"):
        assert not np.any(np.asarray(inputs[nm])), f"{nm} nonzero; unsupported"

    ln_simple = all(
        np.all(np.asarray(inputs[nm], np.float32) == v)
        for nm, v in (("ln1_w", 1.0), ("ln2_w", 1.0), ("lnf_w", 1.0),
                      ("ln1_b", 0.0), ("ln2_b", 0.0), ("lnf_b", 0.0)))

    wqk = np.stack([_pack_stationary(aw[l][:, :2 * C], 12) for l in range(L)]).astype(BF16)
    wv = np.stack([np.ascontiguousarray(
        aw[l][:, 2 * C:].reshape(CB, 128, C).transpose(1, 0, 2).reshape(128, CB * C))
        for l in range(L)]).astype(BF16)
    wproj = np.stack([_pack_stationary(pw[l], CB) for l in range(L)]).astype(BF16)
    wfc = np.stack([_pack_stationary(fw[l], FBL) for l in range(L)]).astype(BF16)
    wmp = np.stack([_pack_stationary(mw[l], CB) for l in range(L)]).astype(BF16)

    lmp = np.zeros((C, VPAD), np.float32)
    lmp[:, :V] = lm
    wlm_halves = []
    for vh in range(2):
        t = lmp[:, vh * VSH:(vh + 1) * VSH].reshape(CB, 128, NVC, 512)
        wlm_halves.append(np.ascontiguousarray(
            t.transpose(1, 2, 0, 3).reshape(128, NVC * CB * 512)).astype(BF16))

    lncols = np.zeros((128, (4 * L + 2) * CB), np.float32)
    names = [("ln1_w", 0), ("ln1_b", 1), ("ln2_w", 2), ("ln2_b", 3)]
    for l in range(L):
        for nm, k in names:
            vec = np.asarray(inputs[nm], np.float32)[l]
            lncols[:, (4 * l + k) * CB:(4 * l + k + 1) * CB] = \
                vec.reshape(CB, 128).T
    lncols[:, 4 * L * CB:(4 * L + 1) * CB] = \
        np.asarray(inputs["lnf_w"], np.float32).reshape(CB, 128).T
    lncols[:, (4 * L + 1) * CB:] = \
        np.asarray(inputs["lnf_b"], np.float32).reshape(CB, 128).T

    p = np.arange(128)[:, None]
    f = np.arange(512)[None, :]
    masks = np.concatenate(
        [(f >= 128 * k + p).astype(np.float32) for k in range(4)],
        axis=1).astype(BF16)

    selc = np.zeros((12, CB * 128), np.float32)
    for cb in range(CB):
        selc[2 * cb, cb * 128:cb * 128 + 64] = 1.0
        selc[2 * cb + 1, cb * 128 + 64:cb * 128 + 128] = 1.0
    selc = selc.astype(BF16)

    h0s = []
    for b in range(B):
        h = wte[x[b]] + wpe[:T]
        hTr = np.ascontiguousarray(
            h.T.reshape(CB, 128, T).transpose(1, 0, 2).reshape(128, CB * T))
        h0s.append(hTr.astype(np.float32))

    in_maps = []
    for c in range(8):
        in_maps.append({
            "h0": h0s[c // 2], "wqk": wqk, "wv": wv, "wproj": wproj,
            "wfc": wfc, "wmp": wmp, "wlm": wlm_halves[c % 2],
            "lnp": lncols, "msk": masks, "selc": selc,
        })
    return in_maps, ln_simple


def kernel(**inputs):
    from concourse import bass_utils
    in_maps, ln_simple = _prep(inputs)
    key = ("nc", ln_simple)
    if key not in _CACHE:
        t0 = time.time()
        _CACHE[key] = _build(ln_simple=ln_simple)
        print(f"[kernel] build+compile {time.time()-t0:.1f}s", file=sys.stderr)
    nc = _CACHE[key]
    res = bass_utils.run_bass_kernel_spmd(nc, in_maps, core_ids=list(range(8)))
    outs = [r["out"].astype(np.float32) for r in res.results]
    full = np.empty((B, T, V), np.float32)
    for b in range(B):
        full[b] = np.concatenate([outs[2 * b], outs[2 * b + 1]], axis=1)[:, :V]
    return full
